# revision 4
# baseline (speedup 1.0000x reference)
import sys
import numpy as np

sys.path.insert(0, "/opt/trn_rl_repo")

from contextlib import ExitStack
import concourse.bass as bass
import concourse.tile as tile
from concourse import bacc, mybir
from concourse.bass_utils import run_bass_kernel_spmd

F32 = mybir.dt.float32
AF = mybir.ActivationFunctionType
OP = mybir.AluOpType
AX = mybir.AxisListType
PI = float(np.pi)

# ---- ANI-1x AEV hyperparameters ----
A = 96            # atoms per conformation
NSPEC = 4
RCR, RCA = 5.2, 3.5
ETAR, ETAA, ZETA = 16.0, 8.0, 32.0
NSHR = 16         # radial shifts: 0.9 + 0.26875*f
SHR0, SHRD = 0.9, 0.26875
SHFA = [0.9, 1.55, 2.2, 2.85]           # 4 angular radial shifts
SHFZ = [(k + 0.5) * PI / 8 for k in range(8)]  # 8 angle shifts
G = 8             # neighbor slots per species group (max observed count is 8)
M = NSPEC * G     # 32 total slots
QPAIRS = [(0, 0), (0, 1), (0, 2), (0, 3), (1, 1), (1, 2), (1, 3),
          (2, 2), (2, 3), (3, 3)]
NQ = len(QPAIRS)  # 10
PB = G * G        # pairs per block (64)
NP = NQ * PB      # 640 pair slots
NA, NZ = 4, 8
BIG = 1.0e12
NEGBIG = -1.0e30
LAM = 0.7071067811865476  # sqrt(2)/2: Lf = ln(lam*cos + lam) = ln(sqrt(2)*fc)

_NC_CACHE = {}


def _build_nc():
    nc = bacc.Bacc("TRN2", target_bir_lowering=False, debug=False, num_devices=8)
    coords = nc.dram_tensor("coords", [A, 3], F32, kind="ExternalInput")
    spf = nc.dram_tensor("spf", [A, 1], F32, kind="ExternalInput")
    outr = nc.dram_tensor("outr", [NSPEC, NSHR * A], F32, kind="ExternalOutput")
    outa = nc.dram_tensor("outa", [A, NQ * NA * NZ], F32, kind="ExternalOutput")

    with tile.TileContext(nc) as tc, ExitStack() as ctx:
        pool = ctx.enter_context(tc.tile_pool(name="p", bufs=1))
        psum = ctx.enter_context(tc.tile_pool(name="ps", bufs=1, space="PSUM"))
        V, S, P = nc.vector, nc.scalar, nc.gpsimd

        # per-partition-scalar constant columns for activation bias
        bt = pool.tile([A, 12], F32)
        bvals = [PI / 2.0, LAM, -SHR0, 0.0, 0.5] + [-2.0 * sa for sa in SHFA]
        for k, v in enumerate(bvals):
            V.memset(bt[:, k:k + 1], v)
        B_PIH, B_LAM, B_SHR, B_ZERO, B_HALF = (bt[:, k:k + 1] for k in range(5))
        B_A = [bt[:, 5 + k:6 + k] for k in range(NA)]

        # ---------- load + broadcast ----------
        cxyz = pool.tile([A, 3], F32)
        nc.sync.dma_start(cxyz[:], coords.ap())
        spcol = pool.tile([A, 1], F32)
        nc.sync.dma_start(spcol[:], spf.ap())
        BC = pool.tile([A, A, 3], F32)          # BC[i,j,c] = coords[j,c]
        nc.sync.dma_start(BC[:], coords.ap().unsqueeze(0).broadcast_to([A, A, 3]))
        spbc = pool.tile([A, A], F32)           # spbc[i,j] = species[j]
        nc.sync.dma_start(spbc[:], spf.ap().rearrange("j o -> o j").broadcast_to([A, A]))

        # ---------- dense pair planes (partition i, free j) ----------
        dx = pool.tile([A, A, 3], F32)          # coord[j]-coord[i]
        for c in range(3):
            V.tensor_scalar_sub(dx[:, :, c], BC[:, :, c], cxyz[:, c:c + 1])
        dxsq = pool.tile([A, A, 3], F32)
        S.activation(dxsq[:], dx[:], AF.Square)
        d2 = pool.tile([A, A], F32)
        V.tensor_reduce(d2[:], dxsq[:], axis=AX.X, op=OP.add)
        nz = pool.tile([A, A], F32)             # excludes self (d=0)
        V.tensor_scalar(nz[:], d2[:], 0.0, None, op0=OP.is_gt)

        # ---------- angular neighbor compaction ----------
        incut = pool.tile([A, A], F32)
        V.tensor_scalar(incut[:], d2[:], RCA * RCA, None, op0=OP.is_lt)
        V.tensor_mul(incut[:], incut[:], nz[:])
        flags = pool.tile([A, NSPEC, A], F32)
        for g in range(NSPEC):
            V.scalar_tensor_tensor(flags[:, g], spbc[:], float(g), incut[:],
                                   op0=OP.is_equal, op1=OP.mult)
        zeros = pool.tile([A, A], F32)
        V.memset(zeros[:], 0.0)
        scans = pool.tile([A, NSPEC, A], F32)   # inclusive count per species
        for g in range(NSPEC):
            V.tensor_tensor_scan(scans[:, g], flags[:, g], zeros[:], 0.0,
                                 op0=OP.add, op1=OP.add)
        mscan = pool.tile([A, NSPEC, A], F32)
        V.tensor_mul(mscan[:], scans[:], flags[:])
        SLOTP = pool.tile([A, G, A], F32)       # value mu+1, bcast over j
        P.iota(SLOTP[:], pattern=[[1, G], [0, A]], base=1, channel_multiplier=0,
               allow_small_or_imprecise_dtypes=True)
        Sel = pool.tile([A, NSPEC, G, A], F32)
        for g in range(NSPEC):
            V.tensor_tensor(Sel[:, g],
                            mscan[:, g].unsqueeze(1).broadcast_to([A, G, A]),
                            SLOTP[:], op=OP.is_equal)
        # gather xyz of selected neighbors: gxyz[i, g, mu, c]
        prod = pool.tile([A, G, 3, A], F32)
        gxyz = pool.tile([A, NSPEC, G, 3], F32)
        BCr = BC[:].rearrange("p j c -> p c j")
        for g in range(NSPEC):
            V.tensor_mul(prod[:],
                         Sel[:, g].unsqueeze(2).broadcast_to([A, G, 3, A]),
                         BCr.unsqueeze(1).broadcast_to([A, G, 3, A]))
            V.tensor_reduce(gxyz[:, g], prod[:], axis=AX.X, op=OP.add)

        # ---------- slot geometry ----------
        gv = gxyz[:].rearrange("p g m c -> p (g m) c")      # (A, M, 3)
        gdx = pool.tile([A, M, 3], F32)
        for c in range(3):
            V.tensor_scalar_sub(gdx[:, :, c], gv[:, :, c], cxyz[:, c:c + 1])
        gsq = pool.tile([A, M, 3], F32)
        S.activation(gsq[:], gdx[:], AF.Square)
        gd2 = pool.tile([A, M], F32)
        V.tensor_reduce(gd2[:], gsq[:], axis=AX.X, op=OP.add)
        padm = pool.tile([A, NSPEC, G], F32)    # 1 for empty (padded) slots
        for g in range(NSPEC):
            V.tensor_scalar(padm[:, g].unsqueeze(2), SLOTP[:, :, 0:1],
                            scans[:, g, A - 1:A], None, op0=OP.is_gt)
        V.scalar_tensor_tensor(gd2[:], padm[:].rearrange("p g m -> p (g m)"),
                               BIG, gd2[:], op0=OP.mult, op1=OP.add)

        # ---------- pair blocks: raw dot / d^2 products ----------
        def blk(t, g1, g2, extra=None):
            # broadcast slot-slices of t (A, M[, k]) to (A, G(mu of g1), G(nu of g2)[, k])
            s1 = t[:, g1 * G:(g1 + 1) * G]
            s2 = t[:, g2 * G:(g2 + 1) * G]
            if extra is None:
                a1 = s1.unsqueeze(2).broadcast_to([A, G, G])
                a2 = s2.unsqueeze(1).broadcast_to([A, G, G])
            else:
                a1 = s1.unsqueeze(2).broadcast_to([A, G, G, extra])
                a2 = s2.unsqueeze(1).broadcast_to([A, G, G, extra])
            return a1, a2

        RD = pool.tile([A, NQ, G, G], F32)      # sum_c gdx_mu*gdx_nu
        prod3 = pool.tile([A, G, G, 3], F32)
        for b, (g1, g2) in enumerate(QPAIRS):
            a1, a2 = blk(gdx[:].rearrange("p m c -> p m c"), g1, g2, extra=3)
            V.tensor_mul(prod3[:], a1, a2)
            V.tensor_reduce(RD[:, b], prod3[:], axis=AX.X, op=OP.add)
        PD2 = pool.tile([A, NQ, G, G], F32)     # gd2_mu * gd2_nu
        for b, (g1, g2) in enumerate(QPAIRS):
            a1, a2 = blk(gd2[:], g1, g2)
            V.tensor_mul(PD2[:, b], a1, a2)
        RDv = RD[:].rearrange("p q a b -> p (q a b)")
        PD2v = PD2[:].rearrange("p q a b -> p (q a b)")
        rd2 = pool.tile([A, NP], F32)
        S.activation(rd2[:], RDv, AF.Square)
        S2 = pool.tile([A, NP], F32)            # (d1 d2)^2 - 0.9025*dot^2
        V.scalar_tensor_tensor(S2[:], rd2[:], -0.9025, PD2v,
                               op0=OP.mult, op1=OP.add)

        # ---------- sqrt table visit ----------
        dist = pool.tile([A, A], F32)
        S.activation(dist[:], d2[:], AF.Sqrt)
        gdist = pool.tile([A, M], F32)
        S.activation(gdist[:], gd2[:], AF.Sqrt)
        braw = pool.tile([A, NP], F32)          # d1*d2*sqrt(1-0.9025 c^2)
        S.activation(braw[:], S2[:], AF.Sqrt)

        # ---------- post-sqrt vector work ----------
        grinv = pool.tile([A, M], F32)
        V.reciprocal(grinv[:], gdist[:])
        GI2 = pool.tile([A, NQ, G, G], F32)
        for b, (g1, g2) in enumerate(QPAIRS):
            a1, a2 = blk(grinv[:], g1, g2)
            V.tensor_mul(GI2[:, b], a1, a2)
        GI2v = GI2[:].rearrange("p q a b -> p (q a b)")
        cN = pool.tile([A, NP], F32)            # raw cos(theta) (pre-0.95)
        V.tensor_mul(cN[:], RDv, GI2v)
        sN = pool.tile([A, NP], F32)            # sqrt(1-(0.95 c)^2)
        V.tensor_mul(sN[:], braw[:], GI2v)
        SD = pool.tile([A, NQ, G, G], F32)      # d1 + d2
        for b, (g1, g2) in enumerate(QPAIRS):
            a1, a2 = blk(gdist[:], g1, g2)
            V.tensor_add(SD[:, b], a1, a2)
        gdmin = pool.tile([A, M], F32)
        V.tensor_scalar_min(gdmin[:], gdist[:], RCA)
        dminr = pool.tile([A, A], F32)
        V.tensor_scalar_min(dminr[:], dist[:], RCR)

        # ---------- trig table visit: cos(pi*d/rc) = sin(pi/2 - pi*d/rc) ----------
        sinr = pool.tile([A, A], F32)
        S.activation(sinr[:], dminr[:], AF.Sin, bias=B_PIH, scale=-PI / RCR)
        gsin = pool.tile([A, M], F32)
        S.activation(gsin[:], gdmin[:], AF.Sin, bias=B_PIH, scale=-PI / RCA)
        V.tensor_scalar_max(gsin[:], gsin[:], -0.99999994)

        # ---------- radial pre-exp (free table set) ----------
        fcr = pool.tile([A, A], F32)
        V.tensor_scalar(fcr[:], sinr[:], 0.5, 0.5, op0=OP.mult, op1=OP.add)
        V.tensor_mul(fcr[:], fcr[:], nz[:])
        SHI = pool.tile([A, NSHR, A], F32)
        P.iota(SHI[:], pattern=[[1, NSHR], [0, A]], base=0, channel_multiplier=0,
               allow_small_or_imprecise_dtypes=True)
        diff = pool.tile([A, NSHR, A], F32)
        V.scalar_tensor_tensor(diff[:], SHI[:], -SHRD,
                               dist[:].unsqueeze(1).broadcast_to([A, NSHR, A]),
                               op0=OP.mult, op1=OP.add)
        rsq = pool.tile([A, NSHR, A], F32)
        S.activation(rsq[:], diff[:], AF.Square, bias=B_SHR, scale=1.0)
        OH = pool.tile([A, NSPEC], F32)
        for s in range(NSPEC):
            V.tensor_scalar(OH[:, s:s + 1], spcol[:], float(s), None,
                            op0=OP.is_equal)

        # ---------- angle factor (free set: Identity) ----------
        TZ = pool.tile([A, NZ, NP], F32)
        for z in range(NZ):
            S.activation(TZ[:, z], sN[:], AF.Identity, bias=B_HALF,
                         scale=0.5 * float(np.sin(SHFZ[z])))
            V.scalar_tensor_tensor(TZ[:, z], cN[:], 0.475 * float(np.cos(SHFZ[z])),
                                   TZ[:, z], op0=OP.mult, op1=OP.add)
        V.tensor_scalar_max(TZ[:], TZ[:], 1e-30)

        # ---------- ln/exp table visit ----------
        gLf = pool.tile([A, M], F32)            # ln(sqrt(2)*fc_A(d))
        S.activation(gLf[:], gsin[:], AF.Ln, bias=B_LAM, scale=LAM)
        S.activation(TZ[:], TZ[:], AF.Ln, bias=B_ZERO, scale=1.0)

        LL = pool.tile([A, NQ, G, G], F32)      # gLf_mu + gLf_nu (+tri mask)
        for b, (g1, g2) in enumerate(QPAIRS):
            a1, a2 = blk(gLf[:], g1, g2)
            V.tensor_add(LL[:, b], a1, a2)
        MU = pool.tile([A, G, G], F32)
        P.iota(MU[:], pattern=[[1, G], [0, G]], base=0, channel_multiplier=0,
               allow_small_or_imprecise_dtypes=True)
        NU = pool.tile([A, G, G], F32)
        P.iota(NU[:], pattern=[[0, G], [1, G]], base=0, channel_multiplier=0,
               allow_small_or_imprecise_dtypes=True)
        TRI = pool.tile([A, G, G], F32)
        V.tensor_tensor(TRI[:], MU[:], NU[:], op=OP.is_ge)
        V.tensor_scalar_mul(TRI[:], TRI[:], NEGBIG)
        for b, (g1, g2) in enumerate(QPAIRS):
            if g1 == g2:
                V.tensor_add(LL[:, b], LL[:, b], TRI[:])

        Qsq = pool.tile([A, NA, NP], F32)
        SDv = SD[:].rearrange("p q a b -> p (q a b)")
        for a in range(NA):
            S.activation(Qsq[:, a], SDv, AF.Square, bias=B_A[a], scale=1.0)
        QL = pool.tile([A, NA, NP], F32)
        LLv = LL[:].rearrange("p q a b -> p (q a b)")
        V.scalar_tensor_tensor(QL[:], Qsq[:], -2.0,
                               LLv.unsqueeze(1).broadcast_to([A, NA, NP]),
                               op0=OP.mult, op1=OP.add)

        # radial exp + matmul reduction (same table set)
        rexp = pool.tile([A, NSHR, A], F32)
        S.activation(rexp[:], rsq[:], AF.Exp, bias=B_ZERO, scale=-ETAR)
        R = pool.tile([A, NSHR, A], F32)
        V.tensor_mul(R[:], rexp[:],
                     fcr[:].unsqueeze(1).broadcast_to([A, NSHR, A]))
        R2 = R[:].rearrange("p f j -> p (f j)")
        psR = psum.tile([NSPEC, NSHR * A], F32)
        for b in range(3):
            nc.tensor.matmul(psR[:, b * 512:(b + 1) * 512], lhsT=OH[:],
                             rhs=R2[:, b * 512:(b + 1) * 512], start=True, stop=True)
        radial_sb = pool.tile([NSPEC, NSHR * A], F32)
        S.activation(radial_sb[:], psR[:], AF.Copy, bias=0.0, scale=0.25)
        nc.sync.dma_start(outr.ap(), radial_sb[:])

        # ---------- ARG = 32*ln(t_z) + QL -> exp -> block-reduce ----------
        BF16 = mybir.dt.bfloat16
        Bout = pool.tile([A, NQ, NA, NZ], F32)
        argbuf = pool.tile([A, 2, NZ, NP], F32)
        expbuf = pool.tile([A, 2, NZ, NP], BF16)
        for a in range(NA):
            ab = argbuf[:, a % 2]
            eb = expbuf[:, a % 2]
            V.scalar_tensor_tensor(ab, TZ[:], 32.0,
                                     QL[:, a].unsqueeze(1).broadcast_to([A, NZ, NP]),
                                     op0=OP.mult, op1=OP.add)
            S.activation(eb, ab, AF.Exp, bias=B_ZERO, scale=1.0)
            V.tensor_reduce(Bout[:, :, a, :].rearrange("p q z -> p z q"),
                            eb.rearrange("p z (q r) -> p z q r", q=NQ),
                            axis=AX.X, op=OP.add)
        nc.sync.dma_start(outa.ap(), Bout[:].rearrange("p q a z -> p (q a z)"))

    nc.compile()
    return nc


def kernel(species, coordinates):
    species = np.asarray(species)
    coordinates = np.asarray(coordinates, dtype=np.float32)
    C = coordinates.shape[0]

    if "nc" not in _NC_CACHE:
        _NC_CACHE["nc"] = _build_nc()
    nc = _NC_CACHE["nc"]

    in_maps = [{"coords": np.ascontiguousarray(coordinates[c]),
                "spf": species[c].astype(np.float32).reshape(A, 1)}
               for c in range(C)]
    res = run_bass_kernel_spmd(nc, in_maps, core_ids=list(range(8))).results

    out = np.empty((C, A, 384), np.float32)
    for c in range(C):
        radial = res[c]["outr"].reshape(NSPEC, NSHR, A).transpose(2, 0, 1)
        out[c, :, :64] = radial.reshape(A, 64)
        out[c, :, 64:] = res[c]["outa"]
    return out
